# revision 10
# baseline (speedup 1.0000x reference)
"""Trainium2 Bass kernel for nn_Attention (B=4, N=2048, C=1024, H=16).

Sharding: 8 cores; core c -> (batch b = c//2, head-group g = c%2 of 8 heads).
Data-parallel on B, tensor-parallel on H.  Each core computes a full-shape
[C, N] (transposed) partial of the output projection for its head slice; the
host transposes, sums the two partials per batch and adds proj_b.

v6 (from the v5 trace: PE 262us active of 313us, full clock mid-kernel,
losses concentrated at startup 17us / qp boundary 8us / tail 16us):

  - DMA issue spread across the SP/Act/DVE HWDGE queues + gpsimd SWDGE,
    ordered by first compute use (wk+xp -> wv/wq/xf-lo -> qp0 bias tiles ->
    xf-hi -> wp -> qp1 bias tiles).  All KC*2 exp-bias tiles are resident
    in SBUF (no mid-kernel bias DMA, no qp-boundary stall).
  - Softmax denominator: DVE reciprocal of the PV ones-row + gpsimd
    partition_broadcast + gpsimd multiply (no DRAM bounce, saves ~4MB of
    DMA and two hops of latency per head).
  - Last attention slot (qp1, t3) runs as two 512-column sub-slots; the
    qs=2 projection units overlap the second sub-slot's attention, so only
    the qs=3 units (8 matmul chains) trail the last normalize.
  - Attention per (q-block, head): ST scores -> ACT exp -> DVE mul by
    exp(bias) -> PV accumulate; pv evacuated to SBUF immediately (split
    ACT/DVE so both banks free in parallel).
  - Dense-GEMM filler (next kT/qT pair, qp0 projection) interleaved into
    the attention slots keeps the PE busy so the HAM clock gate stays at
    2.4GHz.

Mask compaction: keys permuted per batch so unmasked keys come first; only
the first KU (= roundup128(max unmasked count)) keys kept.  Dropped keys are
masked and contribute exactly 0 in the reference too.
"""
import os
import sys

sys.path.insert(0, "/opt/trn_rl_repo")

import numpy as np
import ml_dtypes
from contextlib import ExitStack

import concourse.bass as bass
import concourse.bacc as bacc
import concourse.tile as tile
from concourse import mybir
from concourse.bass_utils import run_bass_kernel_spmd

F32 = mybir.dt.float32
F32R = mybir.dt.float32r
BF16 = mybir.dt.bfloat16
AF = mybir.ActivationFunctionType
NPBF = ml_dtypes.bfloat16

B, N, C, H, D = 4, 2048, 1024, 16, 64
HG = 8            # heads per core
CG = HG * D       # 512: per-core c_out slice of q/k/v and of proj input
P = 128
E = D + 2         # 66: v columns + ones column + pad (4B-aligned bf16 slices)
MASK_VALUE = -65504.0
SCALE = float(D) ** -0.5

_prog_cache = {}


def _ceil_div(a, b):
    return (a + b - 1) // b


def _build(KU, use_qb):
    """Build the SPMD Bass program (same on all 8 cores) for KU kept keys."""
    KC = KU // P               # number of 128-token key chunks
    QB = N // 512              # 4 query blocks of 512

    nc = bacc.Bacc("TRN2", target_bir_lowering=False, debug=False, num_devices=8)
    xT_d = nc.declare_dram_parameter("xT", [C, N], BF16, isOutput=False)
    xpT_d = nc.declare_dram_parameter("xpT", [C, KU], BF16, isOutput=False)
    expb_d = nc.declare_dram_parameter("expbT", [KU, N], BF16, isOutput=False)
    wq_d = nc.declare_dram_parameter("wq", [P, 8 * CG], BF16, isOutput=False)
    wk_d = nc.declare_dram_parameter("wk", [P, 8 * CG], BF16, isOutput=False)
    wv_d = nc.declare_dram_parameter("wv", [P, 8 * CG], BF16, isOutput=False)
    wp_d = nc.declare_dram_parameter("wp", [P, 4 * C], BF16, isOutput=False)
    qb_d = nc.declare_dram_parameter("qb", [CG], F32, isOutput=False)
    vb_d = nc.declare_dram_parameter("vb", [1, CG], F32, isOutput=False)
    ones_d = nc.declare_dram_parameter("ones", [1, P], F32, isOutput=False)
    vones_d = nc.declare_dram_parameter("vones", [P, HG * E], BF16, isOutput=False)
    outp_d = nc.declare_dram_parameter("outp", [C, N], BF16, isOutput=True)

    with ExitStack() as ctx:
        tc = ctx.enter_context(tile.TileContext(nc))
        persist = ctx.enter_context(tc.tile_pool(name="persist", bufs=1))
        const = ctx.enter_context(tc.tile_pool(name="const", bufs=1))

        ones1 = const.tile([1, P], F32R, name="ones1")
        vb_t = const.tile([1, CG], F32R, name="vb_t")
        qb_t = const.tile([P, 4], F32, name="qb_t")
        vo_t = const.tile([P, HG * E], BF16, name="vo_t")

        qTt = [persist.tile([P, N], BF16, name=f"qT{i}") for i in range(4)]
        kTt = [persist.tile([P, KU], BF16, name=f"kT{i}") for i in range(4)]
        vat = [persist.tile([P, HG * E], BF16, name=f"va{i}") for i in range(KC)]
        ott = [persist.tile([P, N], BF16, name=f"ot{i}") for i in range(4)]
        wp_t = persist.tile([P, 4 * C], BF16, name="wp_t")
        wq_t = persist.tile([P, 8 * CG], BF16, name="wq_t")
        wk_t = persist.tile([P, 8 * CG], BF16, name="wk_t")
        wv_t = persist.tile([P, 8 * CG], BF16, name="wv_t")
        xf = [persist.tile([P, N], BF16, name=f"xf{k}") for k in range(8)]
        xp = [persist.tile([P, KU], BF16, name=f"xp{k}") for k in range(8)]

        kblks = [(b0, min(512, KU - b0)) for b0 in range(0, KU, 512)]

        # ---- DMA issue: SP + Act HWDGE queues (+ gpsimd SWDGE for the qp1
        # bias tiles), ordered by first compute use ----
        # sync(SP): wk -> xp (kT/v deps) -> xf lo halves (qT qp0 dep)
        nc.sync.dma_start(wk_t[:], wk_d[:])
        for k in range(8):
            nc.sync.dma_start(xp[k][:], xpT_d[k * P : (k + 1) * P, :])
        for k in range(8):
            nc.sync.dma_start(xf[k][:, 0:1024], xT_d[k * P : (k + 1) * P, 0:1024])
        # scalar(Act): wv/vo (v deps), wq (qT dep), consts, xf hi, wp (late)
        nc.scalar.dma_start(wv_t[:], wv_d[:])
        nc.scalar.dma_start(vo_t[:], vones_d[:])
        nc.scalar.dma_start(wq_t[:], wq_d[:])
        nc.scalar.dma_start(ones1[:], ones_d[:].bitcast(F32R))
        nc.scalar.dma_start(vb_t[:], vb_d[:].bitcast(F32R))
        nc.scalar.dma_start(
            qb_t[:], qb_d[:].rearrange("(m p) -> p m", p=P)
        )

        with tc.tile_pool(name="bsb", bufs=2 * KC) as bpool, tc.tile_pool(
            name="pp", bufs=4
        ) as ppool, tc.tile_pool(name="ovp", bufs=3) as ovpool, tc.tile_pool(
            name="oev2", bufs=3
        ) as oev2, tc.tile_pool(
            name="bcp", bufs=2
        ) as bcpool, tc.tile_pool(
            name="pst", bufs=2, space="PSUM"
        ) as pst, tc.tile_pool(
            name="ppv", bufs=2, space="PSUM"
        ) as ppv, tc.tile_pool(
            name="fps", bufs=2, space="PSUM"
        ) as fps:

            # ---- all exp-bias tiles resident: qp0 split sync/scalar (first
            # chunks via scalar so they land before the sync queue drains
            # xp+xf), qp1 on gpsimd SWDGE ----
            btiles = [[None] * KC for _ in range(2)]
            for kc in range(KC):
                bt = bpool.tile([P, 1024], BF16, name="b_t", tag="bt")
                eng = nc.scalar if kc < 3 else nc.sync
                eng.dma_start(bt[:], expb_d[kc * P : (kc + 1) * P, 0:1024])
                btiles[0][kc] = bt
            # scalar(Act) continues: xf hi halves, wp
            for k in range(8):
                nc.scalar.dma_start(
                    xf[k][:, 1024:N], xT_d[k * P : (k + 1) * P, 1024:N]
                )
            nc.scalar.dma_start(wp_t[:], wp_d[:])
            for kc in range(KC):
                bt = bpool.tile([P, 1024], BF16, name="b_t2", tag="bt")
                nc.gpsimd.dma_start(bt[:], expb_d[kc * P : (kc + 1) * P, 1024:N])
                btiles[1][kc] = bt

            # ---- emit units (each ~1.7-2.1us of PE work through fps) ----
            def emit_kT_block(m, i):
                b0, w = kblks[i]
                ps = fps.tile([P, 512], F32, name="ps_k", tag="fps")
                for kc8 in range(8):
                    lw = wk_t[:, kc8 * CG + m * P : kc8 * CG + (m + 1) * P]
                    nc.tensor.matmul(
                        ps[:, :w],
                        lhsT=lw,
                        rhs=xp[kc8][:, b0 : b0 + w],
                        start=(kc8 == 0),
                        stop=(kc8 == 7),
                    )
                nc.vector.tensor_copy(kTt[m][:, b0 : b0 + w], ps[:, :w])

            def emit_qT_block(m, nb):
                ps = fps.tile([P, 512], F32, name="ps_q", tag="fps")
                for kc8 in range(8):
                    lw = wq_t[:, kc8 * CG + m * P : kc8 * CG + (m + 1) * P]
                    nc.tensor.matmul(
                        ps[:],
                        lhsT=lw,
                        rhs=xf[kc8][:, nb * 512 : (nb + 1) * 512],
                        start=(kc8 == 0),
                        stop=(kc8 == 7),
                    )
                if use_qb:
                    nc.scalar.activation(
                        qTt[m][:, nb * 512 : (nb + 1) * 512],
                        ps[:],
                        AF.Identity,
                        bias=qb_t[:, m : m + 1],
                    )
                else:
                    nc.vector.tensor_copy(
                        qTt[m][:, nb * 512 : (nb + 1) * 512], ps[:]
                    )

            def emit_v_chunk(tm):
                psv = fps.tile([P, CG], F32, name="ps_v", tag="fps")
                for kc8 in range(8):
                    nc.tensor.matmul(
                        psv[:],
                        lhsT=xp[kc8][:, tm * P : (tm + 1) * P],
                        rhs=wv_t[:, kc8 * CG : (kc8 + 1) * CG],
                        start=(kc8 == 0),
                        stop=False,
                    )
                nc.tensor.matmul(
                    psv[:], lhsT=ones1[0:1, :], rhs=vb_t[0:1, :], start=False,
                    stop=True,
                )
                nc.vector.tensor_copy(vat[tm][:], vo_t[:])
                nc.vector.tensor_copy(
                    vat[tm][:].rearrange("p (h e) -> p h e", e=E)[:, :, 0:D],
                    psv[:].rearrange("p (h e) -> p h e", e=D),
                )

            def emit_proj_cq(cm, qs):
                ps = fps.tile([P, 512], F32, name="ps_p", tag="fps")
                for t4 in range(4):
                    lw = wp_t[:, t4 * C + cm * P : t4 * C + (cm + 1) * P]
                    nc.tensor.matmul(
                        ps[:],
                        lhsT=lw,
                        rhs=ott[t4][:, qs * 512 : (qs + 1) * 512],
                        start=(t4 == 0),
                        stop=(t4 == 3),
                    )
                osb = oev2.tile([P, 512], BF16, name="o_sb", tag="osb")
                nc.scalar.activation(osb[:], ps[:], AF.Copy)
                nc.sync.dma_start(
                    outp_d[cm * P : (cm + 1) * P, qs * 512 : (qs + 1) * 512],
                    osb[:],
                )

            def emit_dummy():
                # keeps the HAM activity window busy; no consumers
                ps = fps.tile([P, 512], F32, name="ps_d", tag="fps")
                nc.tensor.matmul(
                    ps[:], lhsT=wp_t[:, 0:P], rhs=xf[0][:, 0:512],
                    start=True, stop=True,
                )

            def normalize(t, po, cols_lo, width, ov, ov_off, last):
                """1/rowsum via DVE recip (into partition 0 of the bcast
                tile) + gpsimd partition-broadcast into partitions 1..D;
                then ott[t][po:po+D, cols] = ov[0:D] * bcast."""
                bcs = bcpool.tile([D, 1024], F32, name="bcs_t", tag="bcs")
                nc.vector.reciprocal(
                    bcs[0:1, 0:width], ov[D : D + 1, ov_off : ov_off + width]
                )
                # broadcast partition 0 over partitions 0..D (the self-copy
                # of partition 0 is benign) so both mul inputs start at 0
                nc.gpsimd.partition_broadcast(
                    bcs[0:D, 0:width], bcs[0:1, 0:width], channels=D
                )
                meng = nc.vector if last else nc.gpsimd
                meng.tensor_mul(
                    ott[t][po : po + D, cols_lo : cols_lo + width],
                    ov[0:D, ov_off : ov_off + width],
                    bcs[0:D, 0:width],
                )

            # ---- pre-attention GEMMs (kT/qT pair 0 + all v) ----
            for i in range(len(kblks)):
                emit_kT_block(0, i)
            for tm in range(KC):
                emit_v_chunk(tm)
            emit_qT_block(0, 0)
            emit_qT_block(0, 1)

            # ---- filler schedule: slot (qp, t) -> list of thunks ----
            # qp1 proj of qp0's columns: 16 units spread over t0..t2 (t3 is
            # the split sub-slot tail).
            qp1_proj_units = [(qs, cm) for cm in range(8) for qs in range(2)]
            qp1_slot_units = [
                qp1_proj_units[0:4],
                qp1_proj_units[4:8],
                qp1_proj_units[8:12],
            ]
            qp1_sub0_units = qp1_proj_units[12:16]

            def filler_for(qp, t):
                th = []
                if qp == 0:
                    if t < 3:
                        m = t + 1
                        for i in range(len(kblks)):
                            th.append(lambda m=m, i=i: emit_kT_block(m, i))
                        th.append(lambda m=m: emit_qT_block(m, 0))
                        th.append(lambda m=m: emit_qT_block(m, 1))
                    if t == 3:
                        # needed right at (qp1, t0)
                        th.append(lambda: emit_qT_block(0, 2))
                        th.append(lambda: emit_qT_block(0, 3))
                        th.append(emit_dummy)
                        th.append(emit_dummy)
                else:
                    if t < 3:
                        # qT(t+1) qp1-half, needed at (qp1, t+1)
                        th.append(lambda m=t + 1: emit_qT_block(m, 2))
                        th.append(lambda m=t + 1: emit_qT_block(m, 3))
                        for qs, cm in qp1_slot_units[t]:
                            th.append(
                                lambda cm=cm, qs=qs: emit_proj_cq(cm, qs)
                            )
                return th

            def attn_head(qp, t, hh, q0, width, qcol_off, btq, filler_sched,
                          step0, last):
                """One head over `width` query columns starting at q0+qcol_off.
                Returns next step counter."""
                h = 2 * t + hh
                po = hh * D
                nj = width // 512
                pvh = [
                    ppv.tile([P, 512], F32, name="pv_t", tag="pv")
                    for _ in range(nj)
                ]
                step = step0
                for kc in range(KC):
                    stt = pst.tile([P, width], F32, name="st_t", tag="stt")
                    lw = kTt[t][po : po + D, kc * P : (kc + 1) * P]
                    for j in range(nj):
                        nc.tensor.matmul(
                            stt[:, j * 512 : (j + 1) * 512],
                            lhsT=lw,
                            rhs=qTt[t][
                                po : po + D,
                                q0 + qcol_off + j * 512 : q0 + qcol_off + (j + 1) * 512,
                            ],
                            start=True,
                            stop=True,
                        )
                    pt = ppool.tile([P, width], BF16, name="p_t", tag="pt")
                    nc.scalar.activation(pt[:], stt[:], AF.Exp)
                    nc.vector.tensor_mul(
                        pt[:], pt[:], btq[kc][:, qcol_off : qcol_off + width]
                    )
                    if kc == 0:
                        # the first PV of a head waits on the previous pv
                        # evacuation: run filler ahead of it so the PE queue
                        # isn't head-of-line blocked
                        for fn in filler_sched.get(step, []):
                            fn()
                    lv = vat[kc][:, h * E : (h + 1) * E]
                    for j in range(nj):
                        nc.tensor.matmul(
                            pvh[j][0:E, :],
                            lhsT=lv,
                            rhs=pt[:, j * 512 : (j + 1) * 512],
                            start=(kc == 0),
                            stop=(kc == KC - 1),
                        )
                    if kc != 0:
                        for fn in filler_sched.get(step, []):
                            fn()
                    step += 1
                # evacuate pv fast, normalize from the SBUF copy.  Split the
                # evacuation across DVE and ACT so both banks free in parallel.
                ov = ovpool.tile([P, 1024], F32, name="ov_t", tag="ov")
                nc.vector.tensor_copy(ov[0:E, 0:512], pvh[0][0:E, :])
                if nj == 2:
                    nc.scalar.activation(
                        ov[0:E, 512:1024], pvh[1][0:E, :], AF.Copy
                    )
                normalize(t, po, q0 + qcol_off, width, ov, 0, last)
                return step

            for qp in range(QB // 2):
                q0 = qp * 1024
                btq = btiles[qp]
                for t in range(4):
                    if qp == 1 and t == 3:
                        break
                    th = filler_for(qp, t)
                    nsteps = 2 * KC
                    sched = {}
                    for i, fn in enumerate(th):
                        if i == 0:
                            step = 0
                        elif i == 1 and len(th) > 1:
                            step = KC
                        else:
                            step = min(
                                nsteps - 1, (i * nsteps) // max(len(th), 1) + 1
                            )
                        sched.setdefault(step, []).append(fn)
                    step = 0
                    for hh in range(2):
                        step = attn_head(
                            qp, t, hh, q0, 1024, 0, btq, sched, step, False
                        )

            # ---- (qp1, t3): two 512-col sub-slots; qs=2 projection overlaps
            # the second sub-slot's attention, only qs=3 trails ----
            q0 = 1024
            btq = btiles[1]
            for sub in range(2):
                sched = {}
                if sub == 0:
                    # leftover qp0-column proj units keep the PE busy while
                    # ACT works through the narrow-tile exps
                    units = qp1_sub0_units
                else:
                    # qs=2 proj units (cols 1024:1536) as filler: spread over
                    # both heads' kc steps
                    units = [(2, cm) for cm in range(8)]
                for i, (qs, cm) in enumerate(units):
                    stp = min(2 * KC - 1, (i * 2 * KC) // len(units) + 1)
                    sched.setdefault(stp, []).append(
                        lambda cm=cm, qs=qs: emit_proj_cq(cm, qs)
                    )
                step = 0
                for hh in range(2):
                    last = sub == 1 and hh == 1
                    step = attn_head(
                        1, 3, hh, q0, 512, sub * 512, btq, sched, step, last
                    )
            # tail: qs=3 units
            for cm in range(8):
                emit_proj_cq(cm, 3)
    nc.finalize()
    return nc


def kernel(
    x=None,
    attention_mask=None,
    attention_bias=None,
    qkv_w=None,
    q_bias=None,
    v_bias=None,
    proj_w=None,
    proj_b=None,
):
    x = np.ascontiguousarray(np.asarray(x, dtype=np.float32))
    mask = np.asarray(attention_mask).astype(bool)
    bias = np.asarray(attention_bias, dtype=np.float32)
    qkv_w = np.asarray(qkv_w, dtype=np.float32)
    q_bias = np.asarray(q_bias, dtype=np.float32)
    v_bias = np.asarray(v_bias, dtype=np.float32)
    proj_w = np.asarray(proj_w, dtype=np.float32)
    proj_b = np.asarray(proj_b, dtype=np.float32)

    assert x.shape == (B, N, C), x.shape

    # --- mask compaction: unmasked keys first, keep KU of them ---
    perms, us = [], []
    for b in range(B):
        perms.append(np.argsort(mask[b], kind="stable"))
        us.append(int((~mask[b]).sum()))
    KU = min(N, max(P, _ceil_div(max(us), P) * P))
    use_qb = bool(np.any(q_bias))

    key = (KU, use_qb)
    if key not in _prog_cache:
        _prog_cache[key] = _build(KU, use_qb)
    nc = _prog_cache[key]

    ones_h = np.ones((1, P), dtype=np.float32)
    vones_h = np.zeros((P, HG * E), dtype=NPBF)
    vones_h.reshape(P, HG, E)[:, :, D] = 1.0
    mv = np.float32(MASK_VALUE)

    per_b = []
    for b in range(B):
        perm = perms[b][:KU]
        xT = np.ascontiguousarray(x[b].T.astype(NPBF))
        xpT = np.ascontiguousarray(x[b][perm].T.astype(NPBF))
        biasT = bias[b].T[perm] + np.where(mask[b][perm], mv, np.float32(0.0))[:, None]
        expbT = np.ascontiguousarray(np.exp(biasT, dtype=np.float32).astype(NPBF))
        per_b.append((xT, xpT, expbT))

    per_g = []
    for g in range(2):
        sl = slice(g * CG, (g + 1) * CG)

        def tile_w(wT, ncols):  # [C_in, ncols] -> [128, (C_in//128)*ncols]
            return np.ascontiguousarray(
                wT.reshape(wT.shape[0] // P, P, ncols)
                .transpose(1, 0, 2)
                .reshape(P, -1)
                .astype(NPBF)
            )

        wq = tile_w((qkv_w[sl, :] * np.float32(SCALE)).T.astype(np.float32), CG)
        wk = tile_w(np.ascontiguousarray(qkv_w[C + g * CG : C + (g + 1) * CG, :].T), CG)
        wv = tile_w(
            np.ascontiguousarray(qkv_w[2 * C + g * CG : 2 * C + (g + 1) * CG, :].T), CG
        )
        wp = tile_w(np.ascontiguousarray(proj_w[:, sl].T), C)
        qb = np.ascontiguousarray(q_bias[sl] * np.float32(SCALE))
        vb = np.ascontiguousarray(v_bias[sl][None, :])
        per_g.append((wq, wk, wv, wp, qb, vb))

    in_maps = []
    for c in range(8):
        b, g = c // 2, c % 2
        xT, xpT, expbT = per_b[b]
        wq, wk, wv, wp, qb, vb = per_g[g]
        in_maps.append(
            {
                "xT": xT,
                "xpT": xpT,
                "expbT": expbT,
                "wq": wq,
                "wk": wk,
                "wv": wv,
                "wp": wp,
                "qb": qb,
                "vb": vb,
                "ones": ones_h,
                "vones": vones_h,
            }
        )

    trace = bool(int(os.environ.get("KBENCH_TRACE", "0")))
    kw = {}
    if trace:
        kw = dict(
            trace=True,
            trace_cores=[
                int(t) for t in os.environ.get("KBENCH_TRACE_CORES", "0").split(",")
            ],
        )
    res = run_bass_kernel_spmd(nc, in_maps, list(range(8)), **kw)
    if trace:
        kernel.last_exec_ns = res.exec_time_ns
        kernel.last_result = res

    out = np.empty((B, N, C), dtype=np.float32)
    for b in range(B):
        outT = res.results[2 * b]["outp"].astype(np.float32) + res.results[
            2 * b + 1
        ]["outp"].astype(np.float32)
        out[b] = outT.T
        out[b] += proj_b[None, :]
    return out


kernel.last_exec_ns = None
kernel.last_result = None


# revision 13
# speedup vs baseline: 1.4349x; 1.4349x over previous
"""Trainium2 Bass kernel for nn_Attention (B=4, N=2048, C=1024, H=16).

Sharding: 8 cores; core c -> (batch b = c//2, head-group g = c%2 of 8 heads).
Data-parallel on B, tensor-parallel on H.  Each core computes a full-shape
[C, N] (transposed) partial of the output projection for its head slice; the
host transposes, sums the two partials per batch and adds proj_b.

v6 (from the v5 trace: PE 262us active of 313us, full clock mid-kernel,
losses concentrated at startup 17us / qp boundary 8us / tail 16us):

  - DMA issue spread across the SP/Act/DVE HWDGE queues + gpsimd SWDGE,
    ordered by first compute use (wk+xp -> wv/wq/xf-lo -> qp0 bias tiles ->
    xf-hi -> wp -> qp1 bias tiles).  All KC*2 exp-bias tiles are resident
    in SBUF (no mid-kernel bias DMA, no qp-boundary stall).
  - Softmax denominator: DVE reciprocal of the PV ones-row + gpsimd
    partition_broadcast + gpsimd multiply (no DRAM bounce, saves ~4MB of
    DMA and two hops of latency per head).
  - Last attention slot (qp1, t3) runs as two 512-column sub-slots; the
    qs=2 projection units overlap the second sub-slot's attention, so only
    the qs=3 units (8 matmul chains) trail the last normalize.
  - Attention per (q-block, head): ST scores -> ACT exp -> DVE mul by
    exp(bias) -> PV accumulate; pv evacuated to SBUF immediately (split
    ACT/DVE so both banks free in parallel).
  - Dense-GEMM filler (next kT/qT pair, qp0 projection) interleaved into
    the attention slots keeps the PE busy so the HAM clock gate stays at
    2.4GHz.

Mask compaction: keys permuted per batch so unmasked keys come first; only
the first KU (= roundup128(max unmasked count)) keys kept.  Dropped keys are
masked and contribute exactly 0 in the reference too.
"""
import os
import sys

sys.path.insert(0, "/opt/trn_rl_repo")

import numpy as np
import ml_dtypes
from contextlib import ExitStack

import concourse.bass as bass
import concourse.bacc as bacc
import concourse.tile as tile
from concourse import mybir
from concourse.bass_utils import run_bass_kernel_spmd

F32 = mybir.dt.float32
F32R = mybir.dt.float32r
BF16 = mybir.dt.bfloat16
AF = mybir.ActivationFunctionType
NPBF = ml_dtypes.bfloat16

B, N, C, H, D = 4, 2048, 1024, 16, 64
HG = 8            # heads per core
CG = HG * D       # 512: per-core c_out slice of q/k/v and of proj input
P = 128
E = D + 2         # 66: v columns + ones column + pad (4B-aligned bf16 slices)
MASK_VALUE = -65504.0
SCALE = float(D) ** -0.5

_prog_cache = {}


def _ceil_div(a, b):
    return (a + b - 1) // b


def _build(KU, use_qb):
    """Build the SPMD Bass program (same on all 8 cores) for KU kept keys."""
    KC = KU // P               # number of 128-token key chunks
    QB = N // 512              # 4 query blocks of 512

    nc = bacc.Bacc("TRN2", target_bir_lowering=False, debug=False, num_devices=8)
    xT_d = nc.declare_dram_parameter("xT", [C, N], BF16, isOutput=False)
    xpT_d = nc.declare_dram_parameter("xpT", [C, KU], BF16, isOutput=False)
    expb_d = nc.declare_dram_parameter("expbT", [KU, N], BF16, isOutput=False)
    wq_d = nc.declare_dram_parameter("wq", [P, 8 * CG], BF16, isOutput=False)
    wk_d = nc.declare_dram_parameter("wk", [P, 8 * CG], BF16, isOutput=False)
    wv_d = nc.declare_dram_parameter("wv", [P, 8 * CG], BF16, isOutput=False)
    wp_d = nc.declare_dram_parameter("wp", [P, 4 * C], BF16, isOutput=False)
    qb_d = nc.declare_dram_parameter("qb", [CG], F32, isOutput=False)
    vb_d = nc.declare_dram_parameter("vb", [1, CG], F32, isOutput=False)
    ones_d = nc.declare_dram_parameter("ones", [1, P], F32, isOutput=False)
    vones_d = nc.declare_dram_parameter("vones", [P, HG * E], BF16, isOutput=False)
    outp_d = nc.declare_dram_parameter("outp", [C, N], BF16, isOutput=True)

    scr_d = nc.dram_tensor("rs_scratch", [20, 1024], F32)

    with ExitStack() as ctx:
        tc = ctx.enter_context(tile.TileContext(nc))
        persist = ctx.enter_context(tc.tile_pool(name="persist", bufs=1))
        const = ctx.enter_context(tc.tile_pool(name="const", bufs=1))

        ones1 = const.tile([1, P], F32R, name="ones1")
        vb_t = const.tile([1, CG], F32R, name="vb_t")
        qb_t = const.tile([P, 4], F32, name="qb_t")
        vo_t = const.tile([P, HG * E], BF16, name="vo_t")

        qTt = [persist.tile([P, N], BF16, name=f"qT{i}") for i in range(4)]
        kTt = [persist.tile([P, KU], BF16, name=f"kT{i}") for i in range(4)]
        vat = [persist.tile([P, HG * E], BF16, name=f"va{i}") for i in range(KC)]
        ott = [persist.tile([P, N], BF16, name=f"ot{i}") for i in range(4)]
        wp_t = persist.tile([P, 4 * C], BF16, name="wp_t")
        wq_t = persist.tile([P, 8 * CG], BF16, name="wq_t")
        wk_t = persist.tile([P, 8 * CG], BF16, name="wk_t")
        wv_t = persist.tile([P, 8 * CG], BF16, name="wv_t")
        xf = [persist.tile([P, N], BF16, name=f"xf{k}") for k in range(8)]
        xp = [persist.tile([P, KU], BF16, name=f"xp{k}") for k in range(8)]

        kblks = [(b0, min(512, KU - b0)) for b0 in range(0, KU, 512)]

        # ---- DMA issue: SP + Act HWDGE queues (+ gpsimd SWDGE for the qp1
        # bias tiles), ordered by first compute use ----
        # sync(SP): wk -> xp (kT/v deps) -> xf lo halves (qT qp0 dep)
        nc.sync.dma_start(wk_t[:], wk_d[:])
        for k in range(8):
            nc.sync.dma_start(xp[k][:], xpT_d[k * P : (k + 1) * P, :])
        for k in range(8):
            nc.sync.dma_start(xf[k][:, 0:1024], xT_d[k * P : (k + 1) * P, 0:1024])
        # scalar(Act): wv/vo (v deps), wq (qT dep), consts, xf hi, wp (late)
        nc.scalar.dma_start(wv_t[:], wv_d[:])
        nc.scalar.dma_start(vo_t[:], vones_d[:])
        nc.scalar.dma_start(wq_t[:], wq_d[:])
        nc.scalar.dma_start(ones1[:], ones_d[:].bitcast(F32R))
        nc.scalar.dma_start(vb_t[:], vb_d[:].bitcast(F32R))
        nc.scalar.dma_start(
            qb_t[:], qb_d[:].rearrange("(m p) -> p m", p=P)
        )

        with tc.tile_pool(name="bsb", bufs=2 * KC) as bpool, tc.tile_pool(
            name="pp", bufs=4
        ) as ppool, tc.tile_pool(name="ovp", bufs=3) as ovpool, tc.tile_pool(
            name="rsp", bufs=4
        ) as rpool, tc.tile_pool(
            name="oev2", bufs=3
        ) as oev2, tc.tile_pool(
            name="bcp", bufs=2
        ) as bcpool, tc.tile_pool(
            name="pst", bufs=2, space="PSUM"
        ) as pst, tc.tile_pool(
            name="ppv", bufs=2, space="PSUM"
        ) as ppv, tc.tile_pool(
            name="fps", bufs=2, space="PSUM"
        ) as fps:

            # ---- all exp-bias tiles resident: qp0 split sync/scalar (first
            # chunks via scalar so they land before the sync queue drains
            # xp+xf), qp1 on gpsimd SWDGE ----
            btiles = [[None] * KC for _ in range(2)]
            for kc in range(KC):
                bt = bpool.tile([P, 1024], BF16, name="b_t", tag="bt")
                eng = nc.scalar if kc < 3 else nc.sync
                eng.dma_start(bt[:], expb_d[kc * P : (kc + 1) * P, 0:1024])
                btiles[0][kc] = bt
            # scalar(Act) continues: xf hi halves, wp
            for k in range(8):
                nc.scalar.dma_start(
                    xf[k][:, 1024:N], xT_d[k * P : (k + 1) * P, 1024:N]
                )
            nc.scalar.dma_start(wp_t[:], wp_d[:])
            for kc in range(KC):
                bt = bpool.tile([P, 1024], BF16, name="b_t2", tag="bt")
                nc.gpsimd.dma_start(bt[:], expb_d[kc * P : (kc + 1) * P, 1024:N])
                btiles[1][kc] = bt

            # ---- emit units (each ~1.7-2.1us of PE work through fps) ----
            def emit_kT_block(m, i):
                b0, w = kblks[i]
                ps = fps.tile([P, 512], F32, name="ps_k", tag="fps")
                for kc8 in range(8):
                    lw = wk_t[:, kc8 * CG + m * P : kc8 * CG + (m + 1) * P]
                    nc.tensor.matmul(
                        ps[:, :w],
                        lhsT=lw,
                        rhs=xp[kc8][:, b0 : b0 + w],
                        start=(kc8 == 0),
                        stop=(kc8 == 7),
                    )
                nc.vector.tensor_copy(kTt[m][:, b0 : b0 + w], ps[:, :w])

            def emit_qT_block(m, nb):
                ps = fps.tile([P, 512], F32, name="ps_q", tag="fps")
                for kc8 in range(8):
                    lw = wq_t[:, kc8 * CG + m * P : kc8 * CG + (m + 1) * P]
                    nc.tensor.matmul(
                        ps[:],
                        lhsT=lw,
                        rhs=xf[kc8][:, nb * 512 : (nb + 1) * 512],
                        start=(kc8 == 0),
                        stop=(kc8 == 7),
                    )
                if use_qb:
                    nc.scalar.activation(
                        qTt[m][:, nb * 512 : (nb + 1) * 512],
                        ps[:],
                        AF.Identity,
                        bias=qb_t[:, m : m + 1],
                    )
                else:
                    nc.vector.tensor_copy(
                        qTt[m][:, nb * 512 : (nb + 1) * 512], ps[:]
                    )

            def emit_v_chunk(tm):
                psv = fps.tile([P, CG], F32, name="ps_v", tag="fps")
                for kc8 in range(8):
                    nc.tensor.matmul(
                        psv[:],
                        lhsT=xp[kc8][:, tm * P : (tm + 1) * P],
                        rhs=wv_t[:, kc8 * CG : (kc8 + 1) * CG],
                        start=(kc8 == 0),
                        stop=False,
                    )
                nc.tensor.matmul(
                    psv[:], lhsT=ones1[0:1, :], rhs=vb_t[0:1, :], start=False,
                    stop=True,
                )
                nc.vector.tensor_copy(vat[tm][:], vo_t[:])
                nc.vector.tensor_copy(
                    vat[tm][:].rearrange("p (h e) -> p h e", e=E)[:, :, 0:D],
                    psv[:].rearrange("p (h e) -> p h e", e=D),
                )

            def emit_proj_cq(cm, qs):
                ps = fps.tile([P, 512], F32, name="ps_p", tag="fps")
                for t4 in range(4):
                    lw = wp_t[:, t4 * C + cm * P : t4 * C + (cm + 1) * P]
                    nc.tensor.matmul(
                        ps[:],
                        lhsT=lw,
                        rhs=ott[t4][:, qs * 512 : (qs + 1) * 512],
                        start=(t4 == 0),
                        stop=(t4 == 3),
                    )
                osb = oev2.tile([P, 512], BF16, name="o_sb", tag="osb")
                nc.scalar.activation(osb[:], ps[:], AF.Copy)
                nc.sync.dma_start(
                    outp_d[cm * P : (cm + 1) * P, qs * 512 : (qs + 1) * 512],
                    osb[:],
                )

            def emit_dummy():
                # keeps the HAM activity window busy; no consumers
                ps = fps.tile([P, 512], F32, name="ps_d", tag="fps")
                nc.tensor.matmul(
                    ps[:], lhsT=wp_t[:, 0:P], rhs=xf[0][:, 0:512],
                    start=True, stop=True,
                )

            norm_it = [0]

            def normalize(t, po, cols_lo, width, ov, ov_off, last):
                """1/rowsum spread across partitions by DMA for a parallel
                DVE reciprocal, DRAM-bounce stride-0 broadcast, multiply on
                the otherwise-idle GPSIMD (DVE for the tail head)."""
                it = norm_it[0]
                norm_it[0] += 1
                wp8 = width // P
                rsw = rpool.tile([P, 8], F32, name="rsw_t", tag="rsw")
                nc.sync.dma_start(
                    rsw[:, 0:wp8], ov[D : D + 1, ov_off : ov_off + width]
                )
                rsw2 = rpool.tile([P, 8], F32, name="rsw2_t", tag="rsw2")
                nc.vector.reciprocal(rsw2[:, 0:wp8], rsw[:, 0:wp8])
                nc.gpsimd.dma_start(scr_d[it : it + 1, 0:width], rsw2[:, 0:wp8])
                bcs = bcpool.tile([D, 1024], F32, name="bcs_t", tag="bcs")
                row = scr_d[it : it + 1, :]
                nc.gpsimd.dma_start(
                    bcs[:, 0:width],
                    bass.AP(
                        tensor=row.tensor,
                        offset=row.offset,
                        ap=[[0, D], [1, width]],
                    ),
                )
                meng = nc.vector if last else nc.gpsimd
                meng.tensor_mul(
                    ott[t][po : po + D, cols_lo : cols_lo + width],
                    ov[0:D, ov_off : ov_off + width],
                    bcs[:, 0:width],
                )

            # ---- pre-attention GEMMs (kT/qT pair 0 + all v) ----
            for i in range(len(kblks)):
                emit_kT_block(0, i)
            for tm in range(KC):
                emit_v_chunk(tm)
            emit_qT_block(0, 0)
            emit_qT_block(0, 1)

            # ---- filler schedule: slot (qp, t) -> list of thunks ----
            # qp1 proj of qp0's columns: 16 units spread over t0..t2 (t3 is
            # the split sub-slot tail).
            qp1_proj_units = [(qs, cm) for cm in range(8) for qs in range(2)]
            qp1_slot_units = [
                qp1_proj_units[0:4],
                qp1_proj_units[4:8],
                qp1_proj_units[8:12],
            ]
            qp1_sub0_units = qp1_proj_units[12:16]

            def filler_for(qp, t):
                th = []
                if qp == 0:
                    if t < 3:
                        m = t + 1
                        for i in range(len(kblks)):
                            th.append(lambda m=m, i=i: emit_kT_block(m, i))
                        th.append(lambda m=m: emit_qT_block(m, 0))
                        th.append(lambda m=m: emit_qT_block(m, 1))
                    if t == 3:
                        # needed right at (qp1, t0)
                        th.append(lambda: emit_qT_block(0, 2))
                        th.append(lambda: emit_qT_block(0, 3))
                        th.append(emit_dummy)
                        th.append(emit_dummy)
                else:
                    if t < 3:
                        # qT(t+1) qp1-half, needed at (qp1, t+1)
                        th.append(lambda m=t + 1: emit_qT_block(m, 2))
                        th.append(lambda m=t + 1: emit_qT_block(m, 3))
                        for qs, cm in qp1_slot_units[t]:
                            th.append(
                                lambda cm=cm, qs=qs: emit_proj_cq(cm, qs)
                            )
                return th

            def attn_head(qp, t, hh, q0, width, qcol_off, btq, filler_sched,
                          step0, last):
                """One head over `width` query columns starting at q0+qcol_off.
                Returns next step counter."""
                h = 2 * t + hh
                po = hh * D
                nj = width // 512
                pvh = [
                    ppv.tile([P, 512], F32, name="pv_t", tag="pv")
                    for _ in range(nj)
                ]
                step = step0
                for kc in range(KC):
                    stt = pst.tile([P, width], F32, name="st_t", tag="stt")
                    lw = kTt[t][po : po + D, kc * P : (kc + 1) * P]
                    for j in range(nj):
                        nc.tensor.matmul(
                            stt[:, j * 512 : (j + 1) * 512],
                            lhsT=lw,
                            rhs=qTt[t][
                                po : po + D,
                                q0 + qcol_off + j * 512 : q0 + qcol_off + (j + 1) * 512,
                            ],
                            start=True,
                            stop=True,
                        )
                    pt = ppool.tile([P, width], BF16, name="p_t", tag="pt")
                    nc.scalar.activation(pt[:], stt[:], AF.Exp)
                    nc.vector.tensor_mul(
                        pt[:], pt[:], btq[kc][:, qcol_off : qcol_off + width]
                    )
                    if kc == 0:
                        # the first PV of a head waits on the previous pv
                        # evacuation: run filler ahead of it so the PE queue
                        # isn't head-of-line blocked
                        for fn in filler_sched.get(step, []):
                            fn()
                    lv = vat[kc][:, h * E : (h + 1) * E]
                    for j in range(nj):
                        nc.tensor.matmul(
                            pvh[j][0:E, :],
                            lhsT=lv,
                            rhs=pt[:, j * 512 : (j + 1) * 512],
                            start=(kc == 0),
                            stop=(kc == KC - 1),
                        )
                    if kc != 0:
                        for fn in filler_sched.get(step, []):
                            fn()
                    step += 1
                # evacuate pv fast, normalize from the SBUF copy.  Split the
                # evacuation across DVE and ACT so both banks free in parallel.
                ov = ovpool.tile([P, 1024], F32, name="ov_t", tag="ov")
                nc.vector.tensor_copy(ov[0:E, 0:512], pvh[0][0:E, :])
                if nj == 2:
                    nc.scalar.activation(
                        ov[0:E, 512:1024], pvh[1][0:E, :], AF.Copy
                    )
                normalize(t, po, q0 + qcol_off, width, ov, 0, last)
                return step

            for qp in range(QB // 2):
                q0 = qp * 1024
                btq = btiles[qp]
                for t in range(4):
                    if qp == 1 and t == 3:
                        break
                    th = filler_for(qp, t)
                    nsteps = 2 * KC
                    sched = {}
                    for i, fn in enumerate(th):
                        if i == 0:
                            step = 0
                        elif i == 1 and len(th) > 1:
                            step = KC
                        else:
                            step = min(
                                nsteps - 1, (i * nsteps) // max(len(th), 1) + 1
                            )
                        sched.setdefault(step, []).append(fn)
                    step = 0
                    for hh in range(2):
                        step = attn_head(
                            qp, t, hh, q0, 1024, 0, btq, sched, step, False
                        )

            # ---- (qp1, t3): two 512-col sub-slots; qs=2 projection overlaps
            # the second sub-slot's attention, only qs=3 trails ----
            q0 = 1024
            btq = btiles[1]
            for sub in range(2):
                sched = {}
                if sub == 0:
                    # leftover qp0-column proj units keep the PE busy while
                    # ACT works through the narrow-tile exps
                    units = qp1_sub0_units
                else:
                    # qs=2 proj units (cols 1024:1536) as filler: spread over
                    # both heads' kc steps
                    units = [(2, cm) for cm in range(8)]
                for i, (qs, cm) in enumerate(units):
                    stp = min(2 * KC - 1, (i * 2 * KC) // len(units) + 1)
                    sched.setdefault(stp, []).append(
                        lambda cm=cm, qs=qs: emit_proj_cq(cm, qs)
                    )
                step = 0
                for hh in range(2):
                    last = sub == 1 and hh == 1
                    step = attn_head(
                        1, 3, hh, q0, 512, sub * 512, btq, sched, step, last
                    )
            # tail: qs=3 units
            for cm in range(8):
                emit_proj_cq(cm, 3)
    nc.finalize()
    return nc


def kernel(
    x=None,
    attention_mask=None,
    attention_bias=None,
    qkv_w=None,
    q_bias=None,
    v_bias=None,
    proj_w=None,
    proj_b=None,
):
    x = np.ascontiguousarray(np.asarray(x, dtype=np.float32))
    mask = np.asarray(attention_mask).astype(bool)
    bias = np.asarray(attention_bias, dtype=np.float32)
    qkv_w = np.asarray(qkv_w, dtype=np.float32)
    q_bias = np.asarray(q_bias, dtype=np.float32)
    v_bias = np.asarray(v_bias, dtype=np.float32)
    proj_w = np.asarray(proj_w, dtype=np.float32)
    proj_b = np.asarray(proj_b, dtype=np.float32)

    assert x.shape == (B, N, C), x.shape

    # --- mask compaction: unmasked keys first, keep KU of them ---
    perms, us = [], []
    for b in range(B):
        perms.append(np.argsort(mask[b], kind="stable"))
        us.append(int((~mask[b]).sum()))
    KU = min(N, max(P, _ceil_div(max(us), P) * P))
    use_qb = bool(np.any(q_bias))

    key = (KU, use_qb)
    if key not in _prog_cache:
        _prog_cache[key] = _build(KU, use_qb)
    nc = _prog_cache[key]

    ones_h = np.ones((1, P), dtype=np.float32)
    vones_h = np.zeros((P, HG * E), dtype=NPBF)
    vones_h.reshape(P, HG, E)[:, :, D] = 1.0
    mv = np.float32(MASK_VALUE)

    per_b = []
    for b in range(B):
        perm = perms[b][:KU]
        xT = np.ascontiguousarray(x[b].T.astype(NPBF))
        xpT = np.ascontiguousarray(x[b][perm].T.astype(NPBF))
        biasT = bias[b].T[perm] + np.where(mask[b][perm], mv, np.float32(0.0))[:, None]
        expbT = np.ascontiguousarray(np.exp(biasT, dtype=np.float32).astype(NPBF))
        per_b.append((xT, xpT, expbT))

    per_g = []
    for g in range(2):
        sl = slice(g * CG, (g + 1) * CG)

        def tile_w(wT, ncols):  # [C_in, ncols] -> [128, (C_in//128)*ncols]
            return np.ascontiguousarray(
                wT.reshape(wT.shape[0] // P, P, ncols)
                .transpose(1, 0, 2)
                .reshape(P, -1)
                .astype(NPBF)
            )

        wq = tile_w((qkv_w[sl, :] * np.float32(SCALE)).T.astype(np.float32), CG)
        wk = tile_w(np.ascontiguousarray(qkv_w[C + g * CG : C + (g + 1) * CG, :].T), CG)
        wv = tile_w(
            np.ascontiguousarray(qkv_w[2 * C + g * CG : 2 * C + (g + 1) * CG, :].T), CG
        )
        wp = tile_w(np.ascontiguousarray(proj_w[:, sl].T), C)
        qb = np.ascontiguousarray(q_bias[sl] * np.float32(SCALE))
        vb = np.ascontiguousarray(v_bias[sl][None, :])
        per_g.append((wq, wk, wv, wp, qb, vb))

    in_maps = []
    for c in range(8):
        b, g = c // 2, c % 2
        xT, xpT, expbT = per_b[b]
        wq, wk, wv, wp, qb, vb = per_g[g]
        in_maps.append(
            {
                "xT": xT,
                "xpT": xpT,
                "expbT": expbT,
                "wq": wq,
                "wk": wk,
                "wv": wv,
                "wp": wp,
                "qb": qb,
                "vb": vb,
                "ones": ones_h,
                "vones": vones_h,
            }
        )

    trace = bool(int(os.environ.get("KBENCH_TRACE", "0")))
    kw = {}
    if trace:
        kw = dict(
            trace=True,
            trace_cores=[
                int(t) for t in os.environ.get("KBENCH_TRACE_CORES", "0").split(",")
            ],
        )
    res = run_bass_kernel_spmd(nc, in_maps, list(range(8)), **kw)
    if trace:
        kernel.last_exec_ns = res.exec_time_ns
        kernel.last_result = res

    out = np.empty((B, N, C), dtype=np.float32)
    for b in range(B):
        outT = res.results[2 * b]["outp"].astype(np.float32) + res.results[
            2 * b + 1
        ]["outp"].astype(np.float32)
        out[b] = outT.T
        out[b] += proj_b[None, :]
    return out


kernel.last_exec_ns = None
kernel.last_result = None


# revision 20
# speedup vs baseline: 1.4615x; 1.0186x over previous
"""Trainium2 Bass kernel for nn_Attention (B=4, N=2048, C=1024, H=16).

Sharding: 8 cores; core c -> (batch b = c//2, head-group g = c%2 of 8 heads).
Data-parallel on B, tensor-parallel on H.  Each core computes a full-shape
[C, N] (transposed) partial of the output projection for its head slice; the
host transposes, sums the two partials per batch and adds proj_b.

v6 (from the v5 trace: PE 262us active of 313us, full clock mid-kernel,
losses concentrated at startup 17us / qp boundary 8us / tail 16us):

  - DMA issue spread across the SP/Act/DVE HWDGE queues + gpsimd SWDGE,
    ordered by first compute use (wk+xp -> wv/wq/xf-lo -> qp0 bias tiles ->
    xf-hi -> wp -> qp1 bias tiles).  All KC*2 exp-bias tiles are resident
    in SBUF (no mid-kernel bias DMA, no qp-boundary stall).
  - Softmax denominator: DVE reciprocal of the PV ones-row + gpsimd
    partition_broadcast + gpsimd multiply (no DRAM bounce, saves ~4MB of
    DMA and two hops of latency per head).
  - Last attention slot (qp1, t3) runs as two 512-column sub-slots; the
    qs=2 projection units overlap the second sub-slot's attention, so only
    the qs=3 units (8 matmul chains) trail the last normalize.
  - Attention per (q-block, head): ST scores -> ACT exp -> DVE mul by
    exp(bias) -> PV accumulate; pv evacuated to SBUF immediately (split
    ACT/DVE so both banks free in parallel).
  - Dense-GEMM filler (next kT/qT pair, qp0 projection) interleaved into
    the attention slots keeps the PE busy so the HAM clock gate stays at
    2.4GHz.

Mask compaction: keys permuted per batch so unmasked keys come first; only
the first KU (= roundup128(max unmasked count)) keys kept.  Dropped keys are
masked and contribute exactly 0 in the reference too.
"""
import os
import sys

sys.path.insert(0, "/opt/trn_rl_repo")

import numpy as np
import ml_dtypes
from contextlib import ExitStack

import concourse.bass as bass
import concourse.bacc as bacc
import concourse.tile as tile
from concourse import mybir
from concourse.bass_utils import run_bass_kernel_spmd

F32 = mybir.dt.float32
F32R = mybir.dt.float32r
BF16 = mybir.dt.bfloat16
AF = mybir.ActivationFunctionType
NPBF = ml_dtypes.bfloat16

B, N, C, H, D = 4, 2048, 1024, 16, 64
HG = 8            # heads per core
CG = HG * D       # 512: per-core c_out slice of q/k/v and of proj input
P = 128
E = D + 2         # 66: v columns + ones column + pad (4B-aligned bf16 slices)
MASK_VALUE = -65504.0
SCALE = float(D) ** -0.5

_prog_cache = {}


def _ceil_div(a, b):
    return (a + b - 1) // b


def _build(KU, use_qb):
    """Build the SPMD Bass program (same on all 8 cores) for KU kept keys."""
    KC = KU // P               # number of 128-token key chunks
    QB = N // 512              # 4 query blocks of 512

    nc = bacc.Bacc("TRN2", target_bir_lowering=False, debug=False, num_devices=8)
    xT_d = nc.declare_dram_parameter("xT", [C, N], BF16, isOutput=False)
    xpT_d = nc.declare_dram_parameter("xpT", [C, KU], BF16, isOutput=False)
    expb_d = nc.declare_dram_parameter("expbT", [KU, N], BF16, isOutput=False)
    wq_d = nc.declare_dram_parameter("wq", [P, 8 * CG], BF16, isOutput=False)
    wk_d = nc.declare_dram_parameter("wk", [P, 8 * CG], BF16, isOutput=False)
    wv_d = nc.declare_dram_parameter("wv", [P, 8 * CG], BF16, isOutput=False)
    wp_d = nc.declare_dram_parameter("wp", [P, 4 * C], BF16, isOutput=False)
    qb_d = nc.declare_dram_parameter("qb", [CG], F32, isOutput=False)
    vb_d = nc.declare_dram_parameter("vb", [1, CG], F32, isOutput=False)
    ones_d = nc.declare_dram_parameter("ones", [1, P], F32, isOutput=False)
    vones_d = nc.declare_dram_parameter("vones", [P, HG * E], BF16, isOutput=False)
    outp_d = nc.declare_dram_parameter("outp", [C, N], BF16, isOutput=True)

    scr_d = nc.dram_tensor("rs_scratch", [20, 1024], F32)

    with ExitStack() as ctx:
        tc = ctx.enter_context(tile.TileContext(nc))
        persist = ctx.enter_context(tc.tile_pool(name="persist", bufs=1))
        const = ctx.enter_context(tc.tile_pool(name="const", bufs=1))

        ones1 = const.tile([1, P], F32R, name="ones1")
        vb_t = const.tile([1, CG], F32R, name="vb_t")
        qb_t = const.tile([P, 4], F32, name="qb_t")
        vo_t = const.tile([P, HG * E], BF16, name="vo_t")

        qTt = [persist.tile([P, N], BF16, name=f"qT{i}") for i in range(4)]
        kTt = [persist.tile([P, KU], BF16, name=f"kT{i}") for i in range(4)]
        vat = [persist.tile([P, HG * E], BF16, name=f"va{i}") for i in range(KC)]
        ott = [persist.tile([P, N], BF16, name=f"ot{i}") for i in range(4)]
        wp_t = persist.tile([P, 4 * C], BF16, name="wp_t")
        wq_t = persist.tile([P, 8 * CG], BF16, name="wq_t")
        wk_t = persist.tile([P, 8 * CG], BF16, name="wk_t")
        wv_t = persist.tile([P, 8 * CG], BF16, name="wv_t")
        xf = [persist.tile([P, N], BF16, name=f"xf{k}") for k in range(8)]
        xp = [persist.tile([P, KU], BF16, name=f"xp{k}") for k in range(8)]

        kblks = [(b0, min(512, KU - b0)) for b0 in range(0, KU, 512)]

        # ---- DMA issue: SP + Act HWDGE queues (+ gpsimd SWDGE for the qp1
        # bias tiles), ordered by first compute use.  wk/wq are laid out
        # m-major on the host so the m=0 slices (first kT/qT units) are the
        # first 256KB chunk. ----
        # sync(SP): wk m0 -> xp lo -> wk m1-3 -> xp hi -> xf lo
        nc.sync.dma_start(wk_t[:, 0:1024], wk_d[:, 0:1024])
        for k in range(8):
            nc.sync.dma_start(xp[k][:, 0:512], xpT_d[k * P : (k + 1) * P, 0:512])
        nc.sync.dma_start(wk_t[:, 1024:4096], wk_d[:, 1024:4096])
        for k in range(8):
            nc.sync.dma_start(
                xp[k][:, 512:KU], xpT_d[k * P : (k + 1) * P, 512:KU]
            )
        for k in range(8):
            nc.sync.dma_start(xf[k][:, 0:1024], xT_d[k * P : (k + 1) * P, 0:1024])
        # scalar(Act): wv/vo (v deps), wq (qT dep), consts, xf hi, wp (late)
        nc.scalar.dma_start(wv_t[:], wv_d[:])
        nc.scalar.dma_start(vo_t[:], vones_d[:])
        nc.scalar.dma_start(wq_t[:, 0:1024], wq_d[:, 0:1024])
        nc.scalar.dma_start(wq_t[:, 1024:4096], wq_d[:, 1024:4096])
        nc.scalar.dma_start(ones1[:], ones_d[:].bitcast(F32R))
        nc.scalar.dma_start(vb_t[:], vb_d[:].bitcast(F32R))
        nc.scalar.dma_start(
            qb_t[:], qb_d[:].rearrange("(m p) -> p m", p=P)
        )

        with tc.tile_pool(name="bsb", bufs=2 * KC) as bpool, tc.tile_pool(
            name="pp", bufs=4
        ) as ppool, tc.tile_pool(name="ovp", bufs=3) as ovpool, tc.tile_pool(
            name="rsp", bufs=4
        ) as rpool, tc.tile_pool(
            name="oev2", bufs=3
        ) as oev2, tc.tile_pool(
            name="bcp", bufs=2
        ) as bcpool, tc.tile_pool(
            name="pst", bufs=2, space="PSUM"
        ) as pst, tc.tile_pool(
            name="ppv", bufs=2, space="PSUM"
        ) as ppv, tc.tile_pool(
            name="fps", bufs=2, space="PSUM"
        ) as fps:

            # ---- all exp-bias tiles resident: qp0 split sync/scalar (first
            # chunks via scalar so they land before the sync queue drains
            # xp+xf), qp1 on gpsimd SWDGE ----
            btiles = [[None] * KC for _ in range(2)]
            for kc in range(KC):
                bt = bpool.tile([P, 1024], BF16, name="b_t", tag="bt")
                eng = nc.scalar if kc < 3 else nc.sync
                eng.dma_start(bt[:], expb_d[kc * P : (kc + 1) * P, 0:1024])
                btiles[0][kc] = bt
            # scalar(Act) continues: xf hi halves, wp
            for k in range(8):
                nc.scalar.dma_start(
                    xf[k][:, 1024:N], xT_d[k * P : (k + 1) * P, 1024:N]
                )
            nc.scalar.dma_start(wp_t[:], wp_d[:])
            for kc in range(KC):
                bt = bpool.tile([P, 1024], BF16, name="b_t2", tag="bt")
                nc.gpsimd.dma_start(bt[:], expb_d[kc * P : (kc + 1) * P, 1024:N])
                btiles[1][kc] = bt

            # ---- emit units (each ~1.7-2.1us of PE work through fps) ----
            def emit_kT_block(m, i):
                b0, w = kblks[i]
                ps = fps.tile([P, 512], F32, name="ps_k", tag="fps")
                for kc8 in range(8):
                    lw = wk_t[:, m * 1024 + kc8 * P : m * 1024 + (kc8 + 1) * P]
                    nc.tensor.matmul(
                        ps[:, :w],
                        lhsT=lw,
                        rhs=xp[kc8][:, b0 : b0 + w],
                        start=(kc8 == 0),
                        stop=(kc8 == 7),
                    )
                nc.vector.tensor_copy(kTt[m][:, b0 : b0 + w], ps[:, :w])

            def emit_qT_block(m, nb):
                ps = fps.tile([P, 512], F32, name="ps_q", tag="fps")
                for kc8 in range(8):
                    lw = wq_t[:, m * 1024 + kc8 * P : m * 1024 + (kc8 + 1) * P]
                    nc.tensor.matmul(
                        ps[:],
                        lhsT=lw,
                        rhs=xf[kc8][:, nb * 512 : (nb + 1) * 512],
                        start=(kc8 == 0),
                        stop=(kc8 == 7),
                    )
                if use_qb:
                    nc.scalar.activation(
                        qTt[m][:, nb * 512 : (nb + 1) * 512],
                        ps[:],
                        AF.Identity,
                        bias=qb_t[:, m : m + 1],
                    )
                else:
                    nc.vector.tensor_copy(
                        qTt[m][:, nb * 512 : (nb + 1) * 512], ps[:]
                    )

            def emit_v_chunk(tm):
                psv = fps.tile([P, CG], F32, name="ps_v", tag="fps")
                for kc8 in range(8):
                    nc.tensor.matmul(
                        psv[:],
                        lhsT=xp[kc8][:, tm * P : (tm + 1) * P],
                        rhs=wv_t[:, kc8 * CG : (kc8 + 1) * CG],
                        start=(kc8 == 0),
                        stop=False,
                    )
                nc.tensor.matmul(
                    psv[:], lhsT=ones1[0:1, :], rhs=vb_t[0:1, :], start=False,
                    stop=True,
                )
                nc.vector.tensor_copy(vat[tm][:], vo_t[:])
                nc.vector.tensor_copy(
                    vat[tm][:].rearrange("p (h e) -> p h e", e=E)[:, :, 0:D],
                    psv[:].rearrange("p (h e) -> p h e", e=D),
                )

            def emit_proj_cq(cm, qs):
                ps = fps.tile([P, 512], F32, name="ps_p", tag="fps")
                for t4 in range(4):
                    lw = wp_t[:, t4 * C + cm * P : t4 * C + (cm + 1) * P]
                    nc.tensor.matmul(
                        ps[:],
                        lhsT=lw,
                        rhs=ott[t4][:, qs * 512 : (qs + 1) * 512],
                        start=(t4 == 0),
                        stop=(t4 == 3),
                    )
                osb = oev2.tile([P, 512], BF16, name="o_sb", tag="osb")
                nc.scalar.activation(osb[:], ps[:], AF.Copy)
                nc.sync.dma_start(
                    outp_d[cm * P : (cm + 1) * P, qs * 512 : (qs + 1) * 512],
                    osb[:],
                )

            def emit_dummy():
                # keeps the HAM activity window busy; no consumers
                ps = fps.tile([P, 512], F32, name="ps_d", tag="fps")
                nc.tensor.matmul(
                    ps[:], lhsT=wp_t[:, 0:P], rhs=xf[0][:, 0:512],
                    start=True, stop=True,
                )

            norm_it = [0]

            def normalize(t, po, cols_lo, width, ov, ov_off, last):
                """1/rowsum spread across partitions by DMA for a parallel
                DVE reciprocal, DRAM-bounce stride-0 broadcast, multiply on
                the otherwise-idle GPSIMD (DVE for the tail head)."""
                it = norm_it[0]
                norm_it[0] += 1
                wp8 = width // P
                rsw = rpool.tile([P, 8], F32, name="rsw_t", tag="rsw")
                nc.sync.dma_start(
                    rsw[:, 0:wp8], ov[D : D + 1, ov_off : ov_off + width]
                )
                rsw2 = rpool.tile([P, 8], F32, name="rsw2_t", tag="rsw2")
                nc.vector.reciprocal(rsw2[:, 0:wp8], rsw[:, 0:wp8])
                nc.gpsimd.dma_start(scr_d[it : it + 1, 0:width], rsw2[:, 0:wp8])
                bcs = bcpool.tile([D, 1024], F32, name="bcs_t", tag="bcs")
                row = scr_d[it : it + 1, :]
                nc.gpsimd.dma_start(
                    bcs[:, 0:width],
                    bass.AP(
                        tensor=row.tensor,
                        offset=row.offset,
                        ap=[[0, D], [1, width]],
                    ),
                )
                meng = nc.vector if last else nc.gpsimd
                meng.tensor_mul(
                    ott[t][po : po + D, cols_lo : cols_lo + width],
                    ov[0:D, ov_off : ov_off + width],
                    bcs[:, 0:width],
                )

            # ---- pre-attention GEMMs (kT/qT pair 0 + all v) ----
            for i in range(len(kblks)):
                emit_kT_block(0, i)
            for tm in range(KC):
                emit_v_chunk(tm)
            emit_qT_block(0, 0)
            emit_qT_block(0, 1)

            # ---- filler schedule: slot (qp, t) -> list of thunks ----
            # qp1 proj of qp0's columns: 16 units spread over t0..t2 (t3 is
            # the split sub-slot tail).
            qp1_proj_units = [(qs, cm) for cm in range(8) for qs in range(2)]
            qp1_slot_units = [
                qp1_proj_units[0:4],
                qp1_proj_units[4:8],
                qp1_proj_units[8:12],
            ]
            qp1_sub0_units = qp1_proj_units[12:16]

            def filler_for(qp, t):
                """Returns [(step, thunk)] — fillers placed early/mid slot so
                their consumers (next slot's scores) never wait on the DVE
                evacuation at a slot boundary."""
                th = []
                if qp == 0:
                    if t < 3:
                        m = t + 1
                        steps = [0, 3, 7]
                        for i in range(len(kblks)):
                            th.append(
                                (steps[i], lambda m=m, i=i: emit_kT_block(m, i))
                            )
                        th.append((10, lambda m=m: emit_qT_block(m, 0)))
                        th.append((13, lambda m=m: emit_qT_block(m, 1)))
                    if t == 3:
                        # needed right at (qp1, t0)
                        th.append((0, lambda: emit_qT_block(0, 2)))
                        th.append((5, lambda: emit_qT_block(0, 3)))
                        th.append((10, emit_dummy))
                        th.append((14, emit_dummy))
                else:
                    if t < 3:
                        # qT(t+1) qp1-half, needed at (qp1, t+1)
                        th.append((0, lambda m=t + 1: emit_qT_block(m, 2)))
                        th.append((3, lambda m=t + 1: emit_qT_block(m, 3)))
                        for i, (qs, cm) in enumerate(qp1_slot_units[t]):
                            th.append(
                                (6 + 3 * i,
                                 lambda cm=cm, qs=qs: emit_proj_cq(cm, qs))
                            )
                return th

            def attn_head(qp, t, hh, q0, width, qcol_off, btq, filler_sched,
                          step0, last):
                """One head over `width` query columns starting at q0+qcol_off.
                Returns next step counter."""
                h = 2 * t + hh
                po = hh * D
                nj = width // 512
                pvh = [
                    ppv.tile([P, 512], F32, name="pv_t", tag="pv")
                    for _ in range(nj)
                ]
                step = step0
                for kc in range(KC):
                    stt = pst.tile([P, width], F32, name="st_t", tag="stt")
                    lw = kTt[t][po : po + D, kc * P : (kc + 1) * P]
                    for j in range(nj):
                        nc.tensor.matmul(
                            stt[:, j * 512 : (j + 1) * 512],
                            lhsT=lw,
                            rhs=qTt[t][
                                po : po + D,
                                q0 + qcol_off + j * 512 : q0 + qcol_off + (j + 1) * 512,
                            ],
                            start=True,
                            stop=True,
                        )
                    pt = ppool.tile([P, width], BF16, name="p_t", tag="pt")
                    nc.scalar.activation(pt[:], stt[:], AF.Exp)
                    nc.vector.tensor_mul(
                        pt[:], pt[:], btq[kc][:, qcol_off : qcol_off + width]
                    )
                    if kc == 0:
                        # the first PV of a head waits on the previous pv
                        # evacuation: run filler ahead of it so the PE queue
                        # isn't head-of-line blocked
                        for fn in filler_sched.get(step, []):
                            fn()
                    lv = vat[kc][:, h * E : (h + 1) * E]
                    for j in range(nj):
                        nc.tensor.matmul(
                            pvh[j][0:E, :],
                            lhsT=lv,
                            rhs=pt[:, j * 512 : (j + 1) * 512],
                            start=(kc == 0),
                            stop=(kc == KC - 1),
                        )
                    if kc != 0:
                        for fn in filler_sched.get(step, []):
                            fn()
                    step += 1
                # evacuate pv fast, normalize from the SBUF copy.  Split the
                # evacuation across DVE and ACT so both banks free in parallel.
                ov = ovpool.tile([P, 1024], F32, name="ov_t", tag="ov")
                nc.vector.tensor_copy(ov[0:E, 0:512], pvh[0][0:E, :])
                if nj == 2:
                    nc.scalar.activation(
                        ov[0:E, 512:1024], pvh[1][0:E, :], AF.Copy
                    )
                normalize(t, po, q0 + qcol_off, width, ov, 0, last)
                return step

            for qp in range(QB // 2):
                q0 = qp * 1024
                btq = btiles[qp]
                for t in range(4):
                    if qp == 1 and t == 3:
                        break
                    sched = {}
                    for stp, fn in filler_for(qp, t):
                        sched.setdefault(min(stp, 2 * KC - 1), []).append(fn)
                    step = 0
                    for hh in range(2):
                        step = attn_head(
                            qp, t, hh, q0, 1024, 0, btq, sched, step, False
                        )

            # ---- (qp1, t3): two 512-col sub-slots; qs=2 projection overlaps
            # the second sub-slot's attention, only qs=3 trails ----
            q0 = 1024
            btq = btiles[1]
            for sub in range(2):
                sched = {}
                if sub == 0:
                    # leftover qp0-column proj units keep the PE busy while
                    # ACT works through the narrow-tile exps
                    steps_units = [
                        (st, u) for st, u in zip((0, 4, 9, 13), qp1_sub0_units)
                    ]
                else:
                    # qs=2 proj units (cols 1024:1536) late in the sub-slot:
                    # they depend on sub0's last normalize (~5us of bounce
                    # latency), so early placement would stall the PE
                    steps_units = [
                        (12 + (i * 6) // 8, (2, cm)) for i, cm in enumerate(range(8))
                    ]
                for stp, (qs, cm) in steps_units:
                    sched.setdefault(min(stp, 2 * KC - 1), []).append(
                        lambda cm=cm, qs=qs: emit_proj_cq(cm, qs)
                    )
                step = 0
                for hh in range(2):
                    last = sub == 1 and hh == 1
                    step = attn_head(
                        1, 3, hh, q0, 512, sub * 512, btq, sched, step, last
                    )
            # tail: qs=3 units
            for cm in range(8):
                emit_proj_cq(cm, 3)
    nc.finalize()
    return nc


def kernel(
    x=None,
    attention_mask=None,
    attention_bias=None,
    qkv_w=None,
    q_bias=None,
    v_bias=None,
    proj_w=None,
    proj_b=None,
):
    x = np.ascontiguousarray(np.asarray(x, dtype=np.float32))
    mask = np.asarray(attention_mask).astype(bool)
    bias = np.asarray(attention_bias, dtype=np.float32)
    qkv_w = np.asarray(qkv_w, dtype=np.float32)
    q_bias = np.asarray(q_bias, dtype=np.float32)
    v_bias = np.asarray(v_bias, dtype=np.float32)
    proj_w = np.asarray(proj_w, dtype=np.float32)
    proj_b = np.asarray(proj_b, dtype=np.float32)

    assert x.shape == (B, N, C), x.shape

    # --- mask compaction: unmasked keys first, keep KU of them ---
    perms, us = [], []
    for b in range(B):
        perms.append(np.argsort(mask[b], kind="stable"))
        us.append(int((~mask[b]).sum()))
    KU = min(N, max(P, _ceil_div(max(us), P) * P))
    use_qb = bool(np.any(q_bias))

    key = (KU, use_qb)
    if key not in _prog_cache:
        _prog_cache[key] = _build(KU, use_qb)
    nc = _prog_cache[key]

    ones_h = np.ones((1, P), dtype=np.float32)
    vones_h = np.zeros((P, HG * E), dtype=NPBF)
    vones_h.reshape(P, HG, E)[:, :, D] = 1.0
    mv = np.float32(MASK_VALUE)

    per_b = []
    for b in range(B):
        perm = perms[b][:KU]
        xT = np.ascontiguousarray(x[b].T.astype(NPBF))
        xpT = np.ascontiguousarray(x[b][perm].T.astype(NPBF))
        biasT = bias[b].T[perm] + np.where(mask[b][perm], mv, np.float32(0.0))[:, None]
        expbT = np.ascontiguousarray(np.exp(biasT, dtype=np.float32).astype(NPBF))
        per_b.append((xT, xpT, expbT))

    per_g = []
    for g in range(2):
        sl = slice(g * CG, (g + 1) * CG)

        def tile_w(wT, ncols):  # [C_in, ncols] -> [128, (C_in//128)*ncols]
            return np.ascontiguousarray(
                wT.reshape(wT.shape[0] // P, P, ncols)
                .transpose(1, 0, 2)
                .reshape(P, -1)
                .astype(NPBF)
            )

        def tile_w_mm(wT):  # m-major: [1024, 512] -> [128, (m)(kc8)(128)]
            return np.ascontiguousarray(
                wT.reshape(8, P, 4, P)
                .transpose(1, 2, 0, 3)
                .reshape(P, -1)
                .astype(NPBF)
            )

        wq = tile_w_mm((qkv_w[sl, :] * np.float32(SCALE)).T.astype(np.float32))
        wk = tile_w_mm(
            np.ascontiguousarray(qkv_w[C + g * CG : C + (g + 1) * CG, :].T).astype(
                np.float32
            )
        )
        wv = tile_w(
            np.ascontiguousarray(qkv_w[2 * C + g * CG : 2 * C + (g + 1) * CG, :].T), CG
        )
        wp = tile_w(np.ascontiguousarray(proj_w[:, sl].T), C)
        qb = np.ascontiguousarray(q_bias[sl] * np.float32(SCALE))
        vb = np.ascontiguousarray(v_bias[sl][None, :])
        per_g.append((wq, wk, wv, wp, qb, vb))

    in_maps = []
    for c in range(8):
        b, g = c // 2, c % 2
        xT, xpT, expbT = per_b[b]
        wq, wk, wv, wp, qb, vb = per_g[g]
        in_maps.append(
            {
                "xT": xT,
                "xpT": xpT,
                "expbT": expbT,
                "wq": wq,
                "wk": wk,
                "wv": wv,
                "wp": wp,
                "qb": qb,
                "vb": vb,
                "ones": ones_h,
                "vones": vones_h,
            }
        )

    trace = bool(int(os.environ.get("KBENCH_TRACE", "0")))
    kw = {}
    if trace:
        kw = dict(
            trace=True,
            trace_cores=[
                int(t) for t in os.environ.get("KBENCH_TRACE_CORES", "0").split(",")
            ],
        )
    res = run_bass_kernel_spmd(nc, in_maps, list(range(8)), **kw)
    if trace:
        kernel.last_exec_ns = res.exec_time_ns
        kernel.last_result = res

    out = np.empty((B, N, C), dtype=np.float32)
    for b in range(B):
        outT = res.results[2 * b]["outp"].astype(np.float32) + res.results[
            2 * b + 1
        ]["outp"].astype(np.float32)
        out[b] = outT.T
        out[b] += proj_b[None, :]
    return out


kernel.last_exec_ns = None
kernel.last_result = None
